# revision 61
# baseline (speedup 1.0000x reference)
"""ResNet BasicBlock (conv3x3-bn-relu-conv3x3-bn-add-relu) on 8 TRN2 cores.

Data-parallel: batch N=64 split into 8 images per core; conv/BN params
replicated. Each 3x3 conv is computed as 9 shifted [128ci x 128co] bf16
matmuls accumulated in PSUM over a zero-padded [C, 58*58] SBUF image layout
(channels on partitions, padded spatial flattened on the free dim). The host
pre-pads x so each image is one contiguous DMA. Matmul rhs uses strided
valid-column access patterns so no PE cycles are spent on pad columns.

Tap order puts a full-coverage tap first so its start=True matmul
initializes the full PSUM chunk; the off-center taps then skip the boundary
output rows and columns whose input is the zero pad (their contribution is
exactly zero), saving ~4%% of PE columns. conv2 additionally splits its 9
taps across two PSUM banks so the residual add overlaps the second bank's
matmuls. BN scales
are folded into the conv weights on the host and the BN biases ride as two
extra bf16 columns of w1t, so no separate parameter DMAs exist. Startup
DMAs are spread across the scalar/gpsimd/sync queues and the PE is kept
busy with warm-up matmuls until they land (any pre-steady-state PE gap
triggers a multi-us HAM downclock). Output is stored bf16 and upcast on
the host.
"""

import numpy as np
from contextlib import ExitStack

import concourse.bacc as bacc
import concourse.mybir as mybir
from concourse.tile import TileContext
from concourse.bass_utils import run_bass_kernel_spmd

F32 = mybir.dt.float32
BF16 = mybir.dt.bfloat16
RELU = mybir.ActivationFunctionType.Relu

N_CORES = 8
N_IMG = 8          # images per core
C = 128            # channels (== partitions)
H = W = 56
HP = WP = 58       # padded spatial
S = HP * WP        # 3364 padded flat size
ALLOC = S + 4      # margins so strided valid-col views stay in bounds
HW = H * W         # 3136
ROW_CHUNKS = [(8 * k, 8) for k in range(7)]  # 448 f32 = one PSUM bank
# last image's conv2 tail: short final chunk so the post-matmul drain
# (stt + act + out DMA) after the very last matmul is small
ROW_CHUNKS_TAIL = [(8 * k, 8) for k in range(6)] + [(48, 6), (54, 2)]
# center tap first: its matmul is full-width and carries start=True, so the
# off-center taps can skip zero-pad boundary rows/cols of the output chunk
TAPS = [4, 0, 1, 2, 3, 5, 6, 7, 8]
N_WARM = 38        # PE HAM warm-up matmuls (no DMA dependency)


def _rhs3(t, start, nrows, ncols):
    """3D [C, nrows, ncols] view of padded tile t at alloc offset `start`."""
    return t[:, start : start + 58 * nrows].rearrange("p (r w) -> p r w", w=58)[
        :, :, 0:ncols
    ]


def _zero_pads_scalar(nc, t, zc):
    """Zero every padded position of a [128, ALLOC] image tile on ScalarE.

    Implemented as ACT copies from a zero constant tile so the output dtype
    stays float32r (the BIR verifier requires every producer feeding an fp32r
    matmul to emit fp32r). alloc index = flat index + 1.
    """
    nc.scalar.copy(t[:, 0:60], zc[:, 0:60])
    pairs = t[:, 58 : 58 + 57 * 58].rearrange("p (r w) -> p r w", w=58)[:, :, 0:2]
    nc.scalar.copy(pairs, zc[:, 0:114].rearrange("p (r w) -> p r w", w=2))
    nc.scalar.copy(t[:, 3307:ALLOC], zc[:, 0 : ALLOC - 3307])


def _conv_half(nc, ps, w_sb, src, r0, rows, k_lo, k_hi, sim_safe):
    """Taps TAPS[k_lo:k_hi] of conv(src) rows [r0, r0+rows) into psum ps.

    Off-center taps skip the boundary output rows/cols whose input is the
    zero pad row/col (contribution exactly zero). The first tap of each
    accumulation group must cover the FULL chunk region (its start=True
    overwrite initializes everything the consumer reads), so conv2's bank B
    ([5,6,7,8], k_lo=5 — conveniently TAPS[k]==k there) runs tap 7 (dh=+1,
    dw=0, full for non-bottom chunks) first, and for the bottom chunk runs
    tap 5 (dh=0) first with its col-skip disabled. Col-skipped psum views
    are 3-level strided, which CoreSim's matmul shape check rejects —
    sim_safe=True disables col-skips (identical math, slightly more cols).
    Tap k in TAPS order = block k of w_sb (host pre-permuted).
    """
    ps3 = ps[:, :].rearrange("p (r w) -> p r w", w=W)
    ks = list(range(k_lo, k_hi))
    bottom = r0 + rows == H
    if k_lo == 5 and not bottom:
        ks = [7, 5, 6, 8]
    for k in ks:
        t = TAPS[k]
        dh, dw = t // 3 - 1, t % 3 - 1
        is_start = k == ks[0]
        row_lo = 1 if (dh == -1 and r0 == 0) else 0
        row_hi = rows - 1 if (dh == 1 and bottom) else rows
        force_full = is_start and k_lo == 5 and bottom
        if sim_safe or force_full:
            col_lo, col_hi = 0, W
        else:
            col_lo = 1 if dw == -1 else 0
            col_hi = W - 1 if dw == 1 else W
        nrows, ncols = row_hi - row_lo, col_hi - col_lo
        a = 58 * (r0 + row_lo + 1 + dh) + (col_lo + 1 + dw) + 1
        nc.tensor.matmul(
            ps3[:, row_lo:row_hi, col_lo:col_hi],
            w_sb[:, k * C : (k + 1) * C],
            _rhs3(src, a, nrows, ncols),
            start=is_start,
            stop=(k == ks[-1]),
        )


def build_module(n_img=N_IMG, sim_safe=False):
    nc = bacc.Bacc()

    x_d = nc.dram_tensor("x", [n_img, C, ALLOC], BF16, kind="ExternalInput")
    # BN scales are folded into the conv weights on the host; the two BN bias
    # vectors ride as 2 extra bf16 columns of w1t so no separate param DMA is
    # needed (a [C,1] DMA degenerates to 128 4-byte packets taking ~3-7us)
    w1_d = nc.dram_tensor("w1t", [C, 9 * C + 64], BF16, kind="ExternalInput")
    w2_d = nc.dram_tensor("w2t", [C, 9 * C], BF16, kind="ExternalInput")
    out_d = nc.dram_tensor("out", [n_img, C, HW], BF16, kind="ExternalOutput")

    with TileContext(nc) as tc, ExitStack() as ctx:
        wpool = ctx.enter_context(tc.tile_pool(name="wpool", bufs=1))
        xpool = ctx.enter_context(tc.tile_pool(name="xpool", bufs=4))
        o1pool = ctx.enter_context(tc.tile_pool(name="o1pool", bufs=2))
        tmppool = ctx.enter_context(tc.tile_pool(name="tmppool", bufs=3))
        opool = ctx.enter_context(tc.tile_pool(name="opool", bufs=3))
        ps1pool = ctx.enter_context(tc.tile_pool(name="ps1", bufs=4, space="PSUM"))
        ps2pool = ctx.enter_context(tc.tile_pool(name="ps2", bufs=4, space="PSUM"))

        w1_sb = wpool.tile([C, 9 * C + 64], BF16, name="w1_sb")
        w2_sb = wpool.tile([C, 9 * C], BF16, name="w2_sb")
        h1_sb = w1_sb[:, 9 * C : 9 * C + 1]
        h2_sb = w1_sb[:, 9 * C + 1 : 9 * C + 2]
        zc = wpool.tile([C, 114], F32, name="zc")
        warm = wpool.tile([C, C], BF16, name="warm")
        nc.vector.memset(zc[:, :], 0.0)
        nc.vector.memset(warm[:, :], 0.0)

        # Warm up the PE HAM clock gate during the initial DMA wait:
        # throwaway matmuls on a locally-zeroed tile, so the ramp starts as
        # soon as the engines come up (~7.5us) instead of after the w1 DMA
        # lands (~11us). Count tuned so the bridge ends right as the w1/x
        # DMAs land, with no PE gap in between (a gap triggers a HAM
        # downclock costing several us of half-speed matmuls).
        psw = ps1pool.tile([C, C], F32, name="psw", tag="ps1_t")
        for i in range(N_WARM):
            nc.tensor.matmul(
                psw[:, :], warm[:, :], warm[:, :],
                start=(i % 8 == 0), stop=(i % 8 == 7 or i == N_WARM - 1),
            )

        # Early DMA rate is only ~100-150 GB/s per queue while the DGE ramps,
        # so the startup-critical tensors are spread across the three
        # engine DMA queues to transfer concurrently: w1's halves on
        # scalar+gpsimd (both land ~10.3us), x on sync, w2 behind w1 on
        # gpsimd (needed only at ~23us).
        nc.scalar.dma_start(w1_sb[:, 0 : 5 * C], w1_d[:, 0 : 5 * C])
        nc.gpsimd.dma_start(w1_sb[:, 5 * C :], w1_d[:, 5 * C :])

        def issue_x(img, cuts=(0, ALLOC // 2, ALLOC)):
            # split the image DMA so the first chunks' matmuls can start
            # before the whole image has landed
            x_pad = xpool.tile([C, ALLOC], BF16, name="x_pad")
            for a, b in zip(cuts, cuts[1:]):
                nc.sync.dma_start(x_pad[:, a:b], x_d[img, :, a:b])
            return x_pad

        x_tiles = [None] * n_img
        # first piece small so image 0 chunk 0 can start asap
        x_tiles[0] = issue_x(0, cuts=(0, 640, 1532, 2424, ALLOC))
        nc.gpsimd.dma_start(w2_sb[:, :], w2_d[:, :])

        for img in range(n_img):
            # prefetch next image's input one iteration ahead so it is never
            # queued behind this image's output DMAs
            if img + 1 < n_img:
                x_tiles[img + 1] = issue_x(img + 1)
            x_pad = x_tiles[img]

            # o1_pad is written only by ScalarE: pad zeroing first, then the
            # per-chunk bn+relu writes of the valid columns.
            o1_pad = o1pool.tile([C, ALLOC], BF16, name="o1_pad")
            _zero_pads_scalar(nc, o1_pad, zc)

            # conv1 + bn1 + relu -> o1_pad
            for r0, rows in ROW_CHUNKS:
                nmm = rows * W
                ps = ps1pool.tile([C, nmm], F32, name="ps1_t")
                _conv_half(nc, ps, w1_sb, x_pad, r0, rows, 0, 9, sim_safe)
                vbase = (1 + r0) * WP + 2
                nc.scalar.activation(
                    _rhs3(o1_pad, vbase, rows, W),
                    ps[:, :].rearrange("p (r w) -> p r w", w=W),
                    RELU, bias=h1_sb, scale=1.0,
                )

            # conv2 + bn2 + residual + relu -> out
            chunks2 = ROW_CHUNKS_TAIL if img == n_img - 1 else ROW_CHUNKS
            if True:
                for r0, rows in chunks2:
                    nmm = rows * W
                    psa = ps2pool.tile([C, nmm], F32, name="ps2_t")
                    _conv_half(nc, psa, w2_sb, o1_pad, r0, rows, 0, 5, sim_safe)
                    psb = ps2pool.tile([C, nmm], F32, name="ps2_t")
                    _conv_half(nc, psb, w2_sb, o1_pad, r0, rows, 5, 9, sim_safe)
                    vbase = (1 + r0) * WP + 2
                    # VectorE: u = bankA + residual (overlaps bank B's
                    # matmuls), then t1 = u + bankB. Each op reads at most
                    # one PSUM operand (ISA limit).
                    u = tmppool.tile([C, nmm], F32, name="u")
                    nc.vector.scalar_tensor_tensor(
                        u[:, :].rearrange("p (r w) -> p r w", w=W),
                        psa[:, :].rearrange("p (r w) -> p r w", w=W),
                        1.0,
                        _rhs3(x_pad, vbase, rows, W),
                        op0=mybir.AluOpType.mult, op1=mybir.AluOpType.add,
                    )
                    t1 = tmppool.tile([C, nmm], F32, name="t1")
                    nc.vector.scalar_tensor_tensor(
                        t1[:, :], psb[:, :], 1.0, u[:, :],
                        op0=mybir.AluOpType.mult, op1=mybir.AluOpType.add,
                    )
                    # ScalarE: out = relu(t1 + shift2), stored bf16
                    outc = opool.tile([C, nmm], BF16, name="outc")
                    nc.scalar.activation(
                        outc[:, :], t1[:, :], RELU, bias=h2_sb, scale=1.0
                    )
                    # gpsimd's DMA completion drain costs ~4-6us after its
                    # last packet, so the final two images' outputs ride the
                    # fast sync queue, putting Q0's drain off the critical
                    # path. The very last chunk issues from ScalarE (the
                    # other HWDGE engine, idle by then) so its issue never
                    # queues behind the previous chunk's on Sync.
                    if img == n_img - 1 and r0 + rows == H:
                        out_eng = nc.scalar
                    elif img >= n_img - 2:
                        out_eng = nc.sync
                    else:
                        out_eng = nc.gpsimd
                    out_eng.dma_start(
                        out_d[img, :, r0 * W : r0 * W + nmm], outc[:, :]
                    )

    nc.compile()
    return nc


EPS = 1e-5


def _prep_params(w1, g1, b1, m1, v1, w2, g2, b2, m2, v2):
    s1 = (g1 / np.sqrt(v1 + EPS)).astype(np.float32)
    h1 = (b1 - m1 * s1).astype(np.float32)
    s2 = (g2 / np.sqrt(v2 + EPS)).astype(np.float32)
    h2 = (b2 - m2 * s2).astype(np.float32)
    import ml_dtypes

    def prep_w(w, s):
        # fold BN scale into output channels; [o,i,kh,kw] -> [i, TAPS k, o]
        wt = (w * s[:, None, None, None]).transpose(1, 2, 3, 0).reshape(C, 9, C)
        return wt[:, TAPS, :].reshape(C, 9 * C).astype(ml_dtypes.bfloat16)

    hh = np.zeros((C, 64), dtype=ml_dtypes.bfloat16)
    hh[:, 0] = h1.astype(ml_dtypes.bfloat16)
    hh[:, 1] = h2.astype(ml_dtypes.bfloat16)
    w1t = np.concatenate([prep_w(w1, s1), hh], axis=1)
    return np.ascontiguousarray(w1t), np.ascontiguousarray(prep_w(w2, s2))


def pad_images(x):
    """[n, C, 56, 56] -> bf16 [n, C, ALLOC] zero-padded 58x58 + margins."""
    import ml_dtypes

    n = x.shape[0]
    buf = np.zeros((n, C, ALLOC), dtype=ml_dtypes.bfloat16)
    v = buf[:, :, 60 : 60 + 58 * 56].reshape(n, C, 56, 58)
    v[:, :, :, :56] = x.astype(ml_dtypes.bfloat16)
    return buf


def kernel(x, w1, g1, b1, m1, v1, w2, g2, b2, m2, v2):
    x = np.asarray(x, dtype=np.float32)
    n = x.shape[0]
    assert n == N_CORES * N_IMG, x.shape
    w1t, w2t = _prep_params(
        np.asarray(w1), np.asarray(g1), np.asarray(b1), np.asarray(m1), np.asarray(v1),
        np.asarray(w2), np.asarray(g2), np.asarray(b2), np.asarray(m2), np.asarray(v2),
    )
    xp = pad_images(x.reshape(n, C, H, W))
    nc = build_module()
    in_maps = []
    for cid in range(N_CORES):
        xs = np.ascontiguousarray(xp[cid * N_IMG : (cid + 1) * N_IMG])
        in_maps.append({"x": xs, "w1t": w1t, "w2t": w2t})
    res = run_bass_kernel_spmd(nc, in_maps, core_ids=list(range(N_CORES)))
    out = np.concatenate(
        [np.asarray(r["out"], dtype=np.float32) for r in res.results], axis=0
    )
    return out.reshape(n, C, H, W)


# revision 62
# speedup vs baseline: 1.0097x; 1.0097x over previous
"""ResNet BasicBlock (conv3x3-bn-relu-conv3x3-bn-add-relu) on 8 TRN2 cores.

Data-parallel: batch N=64 split into 8 images per core; conv/BN params
replicated. Each 3x3 conv is computed as 9 shifted [128ci x 128co] bf16
matmuls accumulated in PSUM over a zero-padded [C, 58*58] SBUF image layout
(channels on partitions, padded spatial flattened on the free dim). The host
pre-pads x so each image is one contiguous DMA. Matmul rhs uses strided
valid-column access patterns so no PE cycles are spent on pad columns.

Tap order puts a full-coverage tap first so its start=True matmul
initializes the full PSUM chunk; the off-center taps then skip the boundary
output rows and columns whose input is the zero pad (their contribution is
exactly zero), saving ~4%% of PE columns. conv2 additionally splits its 9
taps across two PSUM banks so the residual add overlaps the second bank's
matmuls. BN scales
are folded into the conv weights on the host and the BN biases ride as two
extra bf16 columns of w1t, so no separate parameter DMAs exist. Startup
DMAs are spread across the scalar/gpsimd/sync queues and the PE is kept
busy with warm-up matmuls until they land (any pre-steady-state PE gap
triggers a multi-us HAM downclock). Output is stored bf16 and upcast on
the host.
"""

import numpy as np
from contextlib import ExitStack

import concourse.bacc as bacc
import concourse.mybir as mybir
from concourse.tile import TileContext
from concourse.bass_utils import run_bass_kernel_spmd

F32 = mybir.dt.float32
BF16 = mybir.dt.bfloat16
RELU = mybir.ActivationFunctionType.Relu

N_CORES = 8
N_IMG = 8          # images per core
C = 128            # channels (== partitions)
H = W = 56
HP = WP = 58       # padded spatial
S = HP * WP        # 3364 padded flat size
ALLOC = S + 4      # margins so strided valid-col views stay in bounds
HW = H * W         # 3136
ROW_CHUNKS = [(8 * k, 8) for k in range(7)]  # 448 f32 = one PSUM bank
# last image's conv2 tail: short final chunk so the post-matmul drain
# (stt + act + out DMA) after the very last matmul is small
ROW_CHUNKS_TAIL = [(8 * k, 8) for k in range(6)] + [(48, 6), (54, 2)]
# center tap first: its matmul is full-width and carries start=True, so the
# off-center taps can skip zero-pad boundary rows/cols of the output chunk
TAPS = [4, 0, 1, 2, 3, 5, 6, 7, 8]
N_WARM = 38        # PE HAM warm-up matmuls (no DMA dependency)


def _rhs3(t, start, nrows, ncols):
    """3D [C, nrows, ncols] view of padded tile t at alloc offset `start`."""
    return t[:, start : start + 58 * nrows].rearrange("p (r w) -> p r w", w=58)[
        :, :, 0:ncols
    ]


def _zero_pads_scalar(nc, t, zc):
    """Zero every padded position of a [128, ALLOC] image tile on ScalarE.

    Implemented as ACT copies from a zero constant tile so the output dtype
    stays float32r (the BIR verifier requires every producer feeding an fp32r
    matmul to emit fp32r). alloc index = flat index + 1.
    """
    nc.scalar.copy(t[:, 0:60], zc[:, 0:60])
    pairs = t[:, 58 : 58 + 57 * 58].rearrange("p (r w) -> p r w", w=58)[:, :, 0:2]
    nc.scalar.copy(pairs, zc[:, 0:114].rearrange("p (r w) -> p r w", w=2))
    nc.scalar.copy(t[:, 3307:ALLOC], zc[:, 0 : ALLOC - 3307])


def _conv_half(nc, ps, w_sb, src, r0, rows, k_lo, k_hi, sim_safe):
    """Taps TAPS[k_lo:k_hi] of conv(src) rows [r0, r0+rows) into psum ps.

    Off-center taps skip the boundary output rows/cols whose input is the
    zero pad row/col (contribution exactly zero). The first tap of each
    accumulation group must cover the FULL chunk region (its start=True
    overwrite initializes everything the consumer reads), so conv2's bank B
    ([5,6,7,8], k_lo=5 — conveniently TAPS[k]==k there) runs tap 7 (dh=+1,
    dw=0, full for non-bottom chunks) first, and for the bottom chunk runs
    tap 5 (dh=0) first with its col-skip disabled. Col-skipped psum views
    are 3-level strided, which CoreSim's matmul shape check rejects —
    sim_safe=True disables col-skips (identical math, slightly more cols).
    Tap k in TAPS order = block k of w_sb (host pre-permuted).
    """
    ps3 = ps[:, :].rearrange("p (r w) -> p r w", w=W)
    ks = list(range(k_lo, k_hi))
    bottom = r0 + rows == H
    if k_lo == 5 and not bottom:
        ks = [7, 5, 6, 8]
    for k in ks:
        t = TAPS[k]
        dh, dw = t // 3 - 1, t % 3 - 1
        is_start = k == ks[0]
        row_lo = 1 if (dh == -1 and r0 == 0) else 0
        row_hi = rows - 1 if (dh == 1 and bottom) else rows
        force_full = is_start and k_lo == 5 and bottom
        if sim_safe or force_full:
            col_lo, col_hi = 0, W
        else:
            col_lo = 1 if dw == -1 else 0
            col_hi = W - 1 if dw == 1 else W
        nrows, ncols = row_hi - row_lo, col_hi - col_lo
        a = 58 * (r0 + row_lo + 1 + dh) + (col_lo + 1 + dw) + 1
        nc.tensor.matmul(
            ps3[:, row_lo:row_hi, col_lo:col_hi],
            w_sb[:, k * C : (k + 1) * C],
            _rhs3(src, a, nrows, ncols),
            start=is_start,
            stop=(k == ks[-1]),
        )


def build_module(n_img=N_IMG, sim_safe=False):
    nc = bacc.Bacc()

    x_d = nc.dram_tensor("x", [n_img, C, ALLOC], BF16, kind="ExternalInput")
    # BN scales are folded into the conv weights on the host; the two BN bias
    # vectors ride as 2 extra bf16 columns of w1t so no separate param DMA is
    # needed (a [C,1] DMA degenerates to 128 4-byte packets taking ~3-7us)
    w1_d = nc.dram_tensor("w1t", [C, 9 * C + 64], BF16, kind="ExternalInput")
    w2_d = nc.dram_tensor("w2t", [C, 9 * C], BF16, kind="ExternalInput")
    out_d = nc.dram_tensor("out", [n_img, C, HW], BF16, kind="ExternalOutput")

    with TileContext(nc) as tc, ExitStack() as ctx:
        wpool = ctx.enter_context(tc.tile_pool(name="wpool", bufs=1))
        xpool = ctx.enter_context(tc.tile_pool(name="xpool", bufs=4))
        o1pool = ctx.enter_context(tc.tile_pool(name="o1pool", bufs=2))
        tmppool = ctx.enter_context(tc.tile_pool(name="tmppool", bufs=3))
        opool = ctx.enter_context(tc.tile_pool(name="opool", bufs=3))
        ps1pool = ctx.enter_context(tc.tile_pool(name="ps1", bufs=4, space="PSUM"))
        ps2pool = ctx.enter_context(tc.tile_pool(name="ps2", bufs=4, space="PSUM"))

        w1_sb = wpool.tile([C, 9 * C + 64], BF16, name="w1_sb")
        w2_sb = wpool.tile([C, 9 * C], BF16, name="w2_sb")
        h1_sb = w1_sb[:, 9 * C : 9 * C + 1]
        h2_sb = w1_sb[:, 9 * C + 1 : 9 * C + 2]
        zc = wpool.tile([C, 114], F32, name="zc")
        warm = wpool.tile([C, C], BF16, name="warm")
        nc.vector.memset(zc[:, :], 0.0)
        nc.vector.memset(warm[:, :], 0.0)

        # Warm up the PE HAM clock gate during the initial DMA wait:
        # throwaway matmuls on a locally-zeroed tile, so the ramp starts as
        # soon as the engines come up (~7.5us) instead of after the w1 DMA
        # lands (~11us). Count tuned so the bridge ends right as the w1/x
        # DMAs land, with no PE gap in between (a gap triggers a HAM
        # downclock costing several us of half-speed matmuls).
        psw = ps1pool.tile([C, C], F32, name="psw", tag="ps1_t")
        for i in range(N_WARM):
            nc.tensor.matmul(
                psw[:, :], warm[:, :], warm[:, :],
                start=(i % 8 == 0), stop=(i % 8 == 7 or i == N_WARM - 1),
            )

        # Early DMA rate is only ~100-150 GB/s per queue while the DGE ramps,
        # so the startup-critical tensors are spread across the three
        # engine DMA queues to transfer concurrently: w1's halves on
        # scalar+gpsimd (both land ~10.3us), x on sync, w2 behind w1 on
        # gpsimd (needed only at ~23us).
        nc.scalar.dma_start(w1_sb[:, 0 : 5 * C], w1_d[:, 0 : 5 * C])
        nc.gpsimd.dma_start(w1_sb[:, 5 * C :], w1_d[:, 5 * C :])

        def issue_x(img, cuts=(0, ALLOC // 2, ALLOC)):
            # split the image DMA so the first chunks' matmuls can start
            # before the whole image has landed
            x_pad = xpool.tile([C, ALLOC], BF16, name="x_pad")
            for a, b in zip(cuts, cuts[1:]):
                nc.sync.dma_start(x_pad[:, a:b], x_d[img, :, a:b])
            return x_pad

        x_tiles = [None] * n_img
        # first piece small so image 0 chunk 0 can start asap
        x_tiles[0] = issue_x(0, cuts=(0, 640, 1532, 2424, ALLOC))
        nc.gpsimd.dma_start(w2_sb[:, :], w2_d[:, :])

        for img in range(n_img):
            # prefetch next image's input one iteration ahead so it is never
            # queued behind this image's output DMAs
            if img + 1 < n_img:
                x_tiles[img + 1] = issue_x(img + 1)
            x_pad = x_tiles[img]

            # o1_pad is written only by ScalarE: pad zeroing first, then the
            # per-chunk bn+relu writes of the valid columns.
            o1_pad = o1pool.tile([C, ALLOC], BF16, name="o1_pad")
            _zero_pads_scalar(nc, o1_pad, zc)

            # conv1 + bn1 + relu -> o1_pad
            for r0, rows in ROW_CHUNKS:
                nmm = rows * W
                ps = ps1pool.tile([C, nmm], F32, name="ps1_t")
                _conv_half(nc, ps, w1_sb, x_pad, r0, rows, 0, 9, sim_safe)
                vbase = (1 + r0) * WP + 2
                nc.scalar.activation(
                    _rhs3(o1_pad, vbase, rows, W),
                    ps[:, :].rearrange("p (r w) -> p r w", w=W),
                    RELU, bias=h1_sb, scale=1.0,
                )

            # conv2 + bn2 + residual + relu -> out
            chunks2 = ROW_CHUNKS_TAIL if img == n_img - 1 else ROW_CHUNKS
            if True:
                for r0, rows in chunks2:
                    nmm = rows * W
                    psa = ps2pool.tile([C, nmm], F32, name="ps2_t")
                    _conv_half(nc, psa, w2_sb, o1_pad, r0, rows, 0, 5, sim_safe)
                    psb = ps2pool.tile([C, nmm], F32, name="ps2_t")
                    _conv_half(nc, psb, w2_sb, o1_pad, r0, rows, 5, 9, sim_safe)
                    vbase = (1 + r0) * WP + 2
                    # VectorE: u = bankA + residual (overlaps bank B's
                    # matmuls), then t1 = u + bankB. Each op reads at most
                    # one PSUM operand (ISA limit).
                    u = tmppool.tile([C, nmm], F32, name="u")
                    nc.vector.scalar_tensor_tensor(
                        u[:, :].rearrange("p (r w) -> p r w", w=W),
                        psa[:, :].rearrange("p (r w) -> p r w", w=W),
                        1.0,
                        _rhs3(x_pad, vbase, rows, W),
                        op0=mybir.AluOpType.mult, op1=mybir.AluOpType.add,
                    )
                    t1 = tmppool.tile([C, nmm], F32, name="t1")
                    nc.vector.scalar_tensor_tensor(
                        t1[:, :], psb[:, :], 1.0, u[:, :],
                        op0=mybir.AluOpType.mult, op1=mybir.AluOpType.add,
                    )
                    # ScalarE: out = relu(t1 + shift2), stored bf16
                    outc = opool.tile([C, nmm], BF16, name="outc")
                    nc.scalar.activation(
                        outc[:, :], t1[:, :], RELU, bias=h2_sb, scale=1.0
                    )
                    # gpsimd's DMA completion drain costs ~4-6us after its
                    # last packet, so the final two images' outputs ride the
                    # fast sync queue, putting Q0's drain off the critical
                    # path.
                    out_eng = nc.sync if img >= n_img - 2 else nc.gpsimd
                    out_eng.dma_start(
                        out_d[img, :, r0 * W : r0 * W + nmm], outc[:, :]
                    )

    nc.compile()
    return nc


EPS = 1e-5


def _prep_params(w1, g1, b1, m1, v1, w2, g2, b2, m2, v2):
    s1 = (g1 / np.sqrt(v1 + EPS)).astype(np.float32)
    h1 = (b1 - m1 * s1).astype(np.float32)
    s2 = (g2 / np.sqrt(v2 + EPS)).astype(np.float32)
    h2 = (b2 - m2 * s2).astype(np.float32)
    import ml_dtypes

    def prep_w(w, s):
        # fold BN scale into output channels; [o,i,kh,kw] -> [i, TAPS k, o]
        wt = (w * s[:, None, None, None]).transpose(1, 2, 3, 0).reshape(C, 9, C)
        return wt[:, TAPS, :].reshape(C, 9 * C).astype(ml_dtypes.bfloat16)

    hh = np.zeros((C, 64), dtype=ml_dtypes.bfloat16)
    hh[:, 0] = h1.astype(ml_dtypes.bfloat16)
    hh[:, 1] = h2.astype(ml_dtypes.bfloat16)
    w1t = np.concatenate([prep_w(w1, s1), hh], axis=1)
    return np.ascontiguousarray(w1t), np.ascontiguousarray(prep_w(w2, s2))


def pad_images(x):
    """[n, C, 56, 56] -> bf16 [n, C, ALLOC] zero-padded 58x58 + margins."""
    import ml_dtypes

    n = x.shape[0]
    buf = np.zeros((n, C, ALLOC), dtype=ml_dtypes.bfloat16)
    v = buf[:, :, 60 : 60 + 58 * 56].reshape(n, C, 56, 58)
    v[:, :, :, :56] = x.astype(ml_dtypes.bfloat16)
    return buf


def kernel(x, w1, g1, b1, m1, v1, w2, g2, b2, m2, v2):
    x = np.asarray(x, dtype=np.float32)
    n = x.shape[0]
    assert n == N_CORES * N_IMG, x.shape
    w1t, w2t = _prep_params(
        np.asarray(w1), np.asarray(g1), np.asarray(b1), np.asarray(m1), np.asarray(v1),
        np.asarray(w2), np.asarray(g2), np.asarray(b2), np.asarray(m2), np.asarray(v2),
    )
    xp = pad_images(x.reshape(n, C, H, W))
    nc = build_module()
    in_maps = []
    for cid in range(N_CORES):
        xs = np.ascontiguousarray(xp[cid * N_IMG : (cid + 1) * N_IMG])
        in_maps.append({"x": xs, "w1t": w1t, "w2t": w2t})
    res = run_bass_kernel_spmd(nc, in_maps, core_ids=list(range(N_CORES)))
    out = np.concatenate(
        [np.asarray(r["out"], dtype=np.float32) for r in res.results], axis=0
    )
    return out.reshape(n, C, H, W)
